# revision 38
# baseline (speedup 1.0000x reference)
"""Multi-head self-attention with RoPE, sharded over 8 TRN2 NeuronCores.

Sharding: tensor-parallel over heads (2 heads/core) for QKV projections and
attention; an AllToAll redistributes attention outputs from head-sharded to
sequence-sharded so each core computes 1/8 of the output projection rows.

Device-side layout choices (host pre-stages everything):
- x is passed transposed (xt = x.T) so projection matmuls contract naturally.
- Wq/Wk rows are pair-permuted (evens then odds per head) so RoPE becomes
  rotate-half form; the 1/sqrt(hd) score scale is folded into Wq.
- Scores are computed transposed (S^T = K @ Q^T, keys on partitions) so the
  softmax denominator comes free from an ones-column appended to V, and P^T
  feeds the PV matmul with no on-device transpose of P.
- Attention outputs are normalized per chunk BEFORE the AllToAll (approx
  reciprocal + e01-broadcast matmul), so the collective payload is a single
  contiguous bf16 [128, 512] per shard and the consumer side needs no
  normalization stage.
- proj+rope of chunk sc+1 is emitted before attention of chunk sc so the
  DVE rope work overlaps the PE/ACT attention work of the previous chunk.
- All matmuls run as bf16/float32r (full PE rate).

Hardcoded problem shape: B=1, S=4096, D=1024, H=16, hd=64, theta=10000.
"""

import math

import numpy as np

import concourse.bass as bass
import concourse.mybir as mybir
import concourse.tile as tile
from concourse import bacc
from concourse.bass_utils import run_bass_kernel_spmd

N_CORES = 8
D_MODEL = 1024
NUM_HEADS = 16
HEAD_DIM = 64
THETA = 10000.0
P = 128  # partitions; also = 2 heads x 64 dims per core
KD = D_MODEL // 128  # 8 contraction tiles for the projections

F32 = mybir.dt.float32
F32R = mybir.dt.float32r
BF16 = mybir.dt.bfloat16
EXP = mybir.ActivationFunctionType.Exp

ADT = BF16  # attention matmul dtype (x, Wqkv, Q/K, V, P)


def build(seq: int, p12_reps: int = 1, p3_reps: int = 1, parts: str = "full"):
    """Build the SPMD Bass program for sequence length `seq`.

    p12_reps > 1 wraps phases 1+2 (projections + attention) in an on-device
    For_i loop; p3_reps > 1 unrolls phase 3 (A2A + out-proj) — both exist
    for wall-clock timing above the axon dispatch floor. Defaults give the
    normal single-shot kernel.
    """
    CH = min(512, seq)          # free-dim chunk for matmuls / PSUM banks
    NCH = seq // CH             # number of seq chunks
    KB = seq // 128             # key blocks
    KBC = CH // 128             # key blocks per chunk (4 at CH=512)
    SW = seq // N_CORES         # per-core output seq shard
    HALF = N_CORES // 2

    nc = bacc.Bacc("TRN2", num_devices=N_CORES)

    xt = nc.dram_tensor("xt", [D_MODEL, seq], ADT, kind="ExternalInput")
    wq = nc.dram_tensor("wq", [P, D_MODEL], ADT, kind="ExternalInput")
    wk = nc.dram_tensor("wk", [P, D_MODEL], ADT, kind="ExternalInput")
    wv = nc.dram_tensor("wv", [P, D_MODEL], ADT, kind="ExternalInput")
    wo = nc.dram_tensor("wo", [P, KD * D_MODEL], BF16, kind="ExternalInput")
    ctab = nc.dram_tensor("ctab", [P, seq], F32, kind="ExternalInput")
    stab = nc.dram_tensor("stab", [P, seq], F32, kind="ExternalInput")
    dmaskd = nc.dram_tensor("dmask", [P, KBC * CH], BF16,
                            kind="ExternalInput")
    ident = nc.dram_tensor("ident", [P, 128], F32, kind="ExternalInput")
    onesd = nc.dram_tensor("ones", [P, max(KB, 64)], ADT, kind="ExternalInput")
    e01d = nc.dram_tensor("e01", [2, P], F32R, kind="ExternalInput")
    zerod = nc.dram_tensor("zeros", [P, SW], BF16, kind="ExternalInput")
    out_d = nc.dram_tensor("out", [D_MODEL, SW], F32, kind="ExternalOutput")

    with tile.TileContext(nc) as tc:
        with (
            tc.tile_pool(name="const", bufs=1) as cpool,
            tc.tile_pool(name="mats", bufs=1) as mpool,
            tc.tile_pool(name="xt", bufs=2) as xpool,
            tc.tile_pool(name="sc", bufs=2) as spool,
            tc.tile_pool(name="pt", bufs=6) as ptpool,
            tc.tile_pool(name="at", bufs=1) as atpool,
            tc.tile_pool(name="ps", bufs=2, space="PSUM") as pspool,
            tc.tile_pool(name="pss", bufs=2, space="PSUM") as psspool,
            tc.tile_pool(name="dram", bufs=1, space="DRAM") as dpool,
        ):
            # ---- projection weights first: chunk 0 depends on them ----
            w_sb = {}
            for name, src in (("q", wq), ("k", wk), ("v", wv)):
                t = cpool.tile([P, D_MODEL], ADT, tag=f"w{name}")
                nc.sync.dma_start(out=t[:], in_=src[:])
                w_sb[name] = t
            dmask = idn = ones = e01 = None
            wot = []

            def emit_consts():
                """Remaining constants — emitted after chunk 0's input DMAs
                so they don't delay the first projection."""
                nonlocal dmask, idn, ones, e01
                dmask = cpool.tile([P, KBC * CH], BF16, tag="dmask")
                nc.sync.dma_start(out=dmask[:], in_=dmaskd[:])
                idn = cpool.tile([P, 128], F32, tag="idn")
                nc.sync.dma_start(out=idn[:], in_=ident[:])
                ones = cpool.tile([P, max(KB, 64)], ADT, tag="ones")
                nc.sync.dma_start(out=ones[:], in_=onesd[:])
                e01 = {}
                for h in (0, 1):
                    t = cpool.tile([1, P], F32R, tag=f"e01{h}")
                    nc.sync.dma_start(out=t[:], in_=e01d[h:h + 1, :])
                    e01[h] = t
                for e in range(KD):
                    t = cpool.tile([P, D_MODEL], BF16, tag=f"wo{e}")
                    nc.sync.dma_start(out=t[:], in_=wo[:, bass.ts(e, D_MODEL)])
                    wot.append(t)
                # zero the never-written a2a halves once (DRAM->DRAM)
                for s_ in range(N_CORES):
                    dst = a2a_in1 if s_ >= HALF else a2a_in2
                    nc.sync.dma_start(out=dst[s_], in_=zerod[:])
                # ones columns of vnat (cols 64 and 129 of each 130-block)
                vv = vnat[:].rearrange("p (k c) -> p k c", c=130)
                oo = ones[:, 0:KB].rearrange("p (k c) -> p k c", c=1)
                nc.vector.tensor_copy(vv[:, :, 64:65], oo)
                nc.vector.tensor_copy(vv[:, :, 129:130], oo)

            # ---- persistent matrices ----
            qT = mpool.tile([P, seq], ADT, tag="qT")  # rows: 2 heads x 64
            kT = mpool.tile([P, seq], ADT, tag="kT")
            vnat = mpool.tile([P, KB * 130], ADT, tag="vnat")

            a2a_in1 = dpool.tile([N_CORES, P, SW], BF16, tag="a2a_in1")
            a2a_out1 = dpool.tile([N_CORES, P, SW], BF16, tag="a2a_out1")
            a2a_in2 = dpool.tile([N_CORES, P, SW], BF16, tag="a2a_in2")
            a2a_out2 = dpool.tile([N_CORES, P, SW], BF16, tag="a2a_out2")

            def emit_proj_dmas(sc):
                """Issue the input DMAs for chunk sc; returns the tiles."""
                sl = bass.ts(sc, CH)
                xts = []
                for k in range(KD):
                    t = xpool.tile([P, CH], ADT, tag=f"xt{k}",
                                   name=f"xt_{sc}_{k}")
                    nc.sync.dma_start(
                        out=t[:], in_=xt[128 * k:128 * (k + 1), sl]
                    )
                    xts.append(t)
                ct_c = spool.tile([P, CH], F32, tag="ct", name=f"ct_{sc}")
                nc.sync.dma_start(out=ct_c[:], in_=ctab[:, sl])
                st_c = spool.tile([P, CH], F32, tag="st", name=f"st_{sc}")
                nc.sync.dma_start(out=st_c[:], in_=stab[:, sl])
                return xts, ct_c, st_c

            def proj_qk_units(sc, staged):
                """Generator of PE-op units for chunk sc's Q/K projections +
                rope, interleavable between attention blocks. DVE ops are
                emitted inline at their dependency points."""
                sl = bass.ts(sc, CH)
                xts, ct_c, st_c = staged
                if parts == "dma":
                    return
                sw_c = {}
                for nm in ("qs", "ks"):
                    sw_c[nm] = spool.tile([P, CH], F32, tag=nm,
                                          name=f"sw_{sc}_{nm}")
                for name, dst in (("q", qT[:, sl]), ("k", kT[:, sl])):
                    ps = pspool.tile([P, CH], F32, tag="mm",
                                     name=f"proj_{sc}_{name}")
                    for k in range(KD):
                        nc.tensor.matmul(
                            ps[:],
                            w_sb[name][:, bass.ts(k, 128)],
                            xts[k][:],
                            start=(k == 0),
                            stop=(k == KD - 1),
                        )
                        if k % 2 == 1:
                            yield
                    nc.vector.tensor_copy(dst, ps[:])
                    if parts == "proj":
                        continue
                    # rope: mat = mat*cos + swapped*sin (swapped halves via
                    # DVE copies)
                    mat = qT if name == "q" else kT
                    eng = nc.vector
                    swc = sw_c["qs" if name == "q" else "ks"]
                    for h in (0, 1):
                        for half in (0, 1):
                            d0 = 64 * h + 32 * half
                            s0 = 64 * h + 32 * (1 - half)
                            eng.tensor_copy(
                                swc[d0:d0 + 32, :], mat[s0:s0 + 32, sl]
                            )
                    tm = spool.tile([P, CH], F32, tag=f"tmp_{name}",
                                    name=f"tmp_{sc}_{name}")
                    eng.tensor_mul(tm[:], swc[:], st_c[:])
                    eng.tensor_mul(mat[:, sl], mat[:, sl], ct_c[:])
                    eng.tensor_add(mat[:, sl], mat[:, sl], tm[:])

            def proj_v_units(sc, staged):
                """V projection + per-block transpose into vnat for chunk sc
                (needed only by chunk sc's own diagonal PV, so it drains
                later than the Q/K units)."""
                xts, _, _ = staged
                if parts == "dma":
                    return
                vt_c = spool.tile([P, CH], F32, tag="vt", name=f"vt_{sc}")
                ps = pspool.tile([P, CH], F32, tag="mm",
                                 name=f"proj_{sc}_v")
                for k in range(KD):
                    nc.tensor.matmul(
                        ps[:],
                        w_sb["v"][:, bass.ts(k, 128)],
                        xts[k][:],
                        start=(k == 0),
                        stop=(k == KD - 1),
                    )
                    if k % 2 == 1:
                        yield
                nc.vector.tensor_copy(vt_c[:], ps[:])
                if parts in ("proj", "rope"):
                    return
                for j in range(KBC):
                    kb = sc * KBC + j
                    pst = pspool.tile([P, CH], F32, tag="mm",
                                      name=f"vtr_{kb}")
                    nc.tensor.transpose(
                        pst[:, 0:128], vt_c[:, bass.ts(j, 128)], idn[:]
                    )
                    nc.vector.tensor_copy(
                        vnat[:, 130 * kb:130 * kb + 64], pst[:, 0:64]
                    )
                    nc.vector.tensor_copy(
                        vnat[:, 130 * kb + 65:130 * kb + 129],
                        pst[:, 64:128]
                    )
                    yield

            # Global queue of ((chunk, kind), proj-unit generator) in need
            # order: kind 0 = Q/K+rope (needed when attn(chunk) starts),
            # kind 1 = V+transpose (needed at attn(chunk)'s diagonal).
            # Attention blocks pump one PE unit per block so projection work
            # spreads evenly instead of clumping at chunk boundaries.
            proj_queue = []

            def pump(n=1):
                while n > 0 and proj_queue:
                    try:
                        next(proj_queue[0][1])
                        n -= 1
                    except StopIteration:
                        proj_queue.pop(0)

            def ensure_proj_done(key):
                while proj_queue and proj_queue[0][0] <= key:
                    for _ in proj_queue[0][1]:
                        pass
                    proj_queue.pop(0)

            def drain_all():
                ensure_proj_done((NCH, 1))

            def emit_attn_chunk(qc, prev_epi=None):
                """Attention for query chunk qc (needs proj chunks 0..qc).

                Per key block: S^T for both heads lands in one [128, 2*CH]
                PSUM tile ([0:CH]=h0, [CH:2CH]=h1) so a single wide exp
                covers both heads. Diagonal-band blocks are processed FIRST
                (their masking runs on GPSIMD and gets latency-hidden behind
                the non-diagonal tail of the PV accumulation). Chunk qc+1's
                projection PE units (`feeder`) are pumped between blocks to
                fill TensorE slack while ScalarE streams the exps.
                """
                kbmax = (qc + 1) * KBC
                psu = {}
                for h in (0, 1):
                    psu[h] = pspool.tile([65, CH], F32, tag="u",
                                         name=f"psu_{qc}_{h}")
                # old blocks first: they only need this chunk's roped Q, so
                # attention can start before rope-K of this chunk finishes;
                # the diagonal band comes last (its GPSIMD masks still hide
                # behind the deferred-PV pipeline)
                kb_order = (list(range(0, kbmax - KBC))
                            + list(range(kbmax - KBC, kbmax)))

                def emit_pv(kb, ki, pt, qoff):
                    for h in (0, 1):
                        nc.tensor.matmul(
                            psu[h][:, qoff:CH],
                            vnat[:, 130 * kb + 65 * h:
                                 130 * kb + 65 * (h + 1)],
                            pt[:, CH * h + qoff:CH * (h + 1)],
                            start=(ki == 0),
                            stop=(ki == kbmax - 1),
                        )

                pending = None  # deferred-one-step PV: (kb, ki, pt, qoff)
                for ki, kb in enumerate(kb_order):
                    j = kb - (kbmax - KBC)  # diag index if >= 0
                    if j == 0:
                        # own-chunk V/vnat needed from the first diag PV on
                        ensure_proj_done((qc, 1))
                    # diag block j only contributes to queries >= 128j; skip
                    # the fully-masked left region in scores, exp, and PV
                    qoff = 128 * j if j > 0 else 0
                    pss = psspool.tile([P, 2 * CH], F32, tag="s",
                                       name=f"sc_{qc}_{kb}")
                    for h in (0, 1):
                        nc.tensor.matmul(
                            pss[:, CH * h + qoff:CH * (h + 1)],
                            kT[64 * h:64 * (h + 1), bass.ts(kb, 128)],
                            qT[64 * h:64 * (h + 1),
                               CH * qc + qoff:CH * (qc + 1)],
                            start=True,
                            stop=True,
                        )
                    if parts == "attn_sc":
                        pump()
                        continue
                    pt = ptpool.tile([P, 2 * CH], ADT, tag="pt",
                                     name=f"pt_{qc}_{kb}")
                    if qoff == 0:
                        nc.scalar.activation(pt[:], pss[:], EXP)
                    else:
                        for h in (0, 1):
                            nc.scalar.activation(
                                pt[:, CH * h + qoff:CH * (h + 1)],
                                pss[:, CH * h + qoff:CH * (h + 1)], EXP)
                    if j >= 0:
                        # triangle mask on the [128j, 128j+128) query range
                        for h in (0, 1):
                            nc.gpsimd.tensor_mul(
                                pt[:, CH * h + 128 * j:
                                   CH * h + 128 * (j + 1)],
                                pt[:, CH * h + 128 * j:
                                   CH * h + 128 * (j + 1)],
                                dmask[:, CH * j + 128 * j:
                                      CH * j + 128 * (j + 1)],
                            )
                    if parts == "attn_s":
                        pump()
                        if ki == 1 and prev_epi is not None:
                            prev_epi()
                            prev_epi = None
                        continue
                    if pending is not None:
                        emit_pv(*pending)
                    pending = (kb, ki, pt, qoff)
                    pump()
                    if ki == 1 and prev_epi is not None:
                        # previous chunk's epilogue, deferred past this
                        # chunk's first scores so ScalarE never starves on
                        # the sums->reciprocal->broadcast chain
                        prev_epi()
                        prev_epi = None
                if prev_epi is not None:
                    prev_epi()
                if parts not in ("attn_s", "attn_sc") and pending is not None:
                    emit_pv(*pending)
                if parts in ("attn_s", "attn_sc", "attn_pv"):
                    return None

                def epilogue():
                    # normalize U by the softmax sums (row 64 of each psu)
                    # and ship one contiguous bf16 [128, SW] per shard; the
                    # psu-releasing copies come first
                    from concourse.dve_ops import (
                        RECIP_APPROX_FAST_CONSTS as _RC,
                        RECIPROCAL_APPROX_FAST as _RA,
                    )
                    sms = []
                    for h in (0, 1):
                        sm = spool.tile([1, CH], F32, tag=f"sm{h}",
                                        name=f"sm_{qc}_{h}")
                        nc.vector.tensor_copy(sm[:], psu[h][64:65, :])
                        sms.append(sm)
                    uu = spool.tile([P, CH], F32, tag="uu", name=f"uu_{qc}")
                    for h in (0, 1):
                        nc.vector.tensor_copy(
                            uu[64 * h:64 * (h + 1), :], psu[h][0:64, :]
                        )
                    rb = pspool.tile([P, CH], F32, tag="mm", name=f"rb_{qc}")
                    for h in (0, 1):
                        rs = spool.tile([1, CH], F32R, tag=f"rs{h}",
                                        name=f"rs_{qc}_{h}")
                        nc.vector._custom_dve(
                            _RA, out=rs[:], in0=sms[h][:],
                            s0=_RC["s0"], s1=_RC["s1"], imm2=_RC["imm2"],
                        )
                        nc.tensor.matmul(rb[:], e01[h][:], rs[:],
                                         start=(h == 0), stop=(h == 1))
                    ut = ptpool.tile([P, CH], BF16, tag="ut", name=f"ut_{qc}")
                    nc.vector.tensor_mul(ut[:], uu[:], rb[:])
                    tgt = a2a_in1 if qc < HALF else a2a_in2
                    nc.sync.dma_start(out=tgt[qc], in_=ut[:])

                return epilogue

            def emit_at_loads(ab_out, tiles, tag):
                for i in range(N_CORES):
                    at = atpool.tile([P, SW], BF16, tag=f"{tag}{i}",
                                     name=f"{tag}_{i}")
                    nc.sync.dma_start(out=at[:], in_=ab_out[i])
                    tiles.append(at)

            def emit_p12(at1, staged0=None):
                # proj is staged 2 chunks ahead of attention; its PE units
                # go through the global proj_queue, pumped one per attention
                # block so projection work spreads evenly over the phase and
                # the ScalarE exp stream stays dense.
                attn_on = parts not in ("dma", "proj", "rope")

                def stage(sc):
                    staged = emit_proj_dmas(sc)
                    proj_queue.append(((sc, 0), proj_qk_units(sc, staged)))
                    proj_queue.append(((sc, 1), proj_v_units(sc, staged)))

                if staged0 is not None:
                    proj_queue.append(((0, 0), proj_qk_units(0, staged0)))
                    proj_queue.append(((0, 1), proj_v_units(0, staged0)))
                else:
                    stage(0)
                if NCH > 1:
                    stage(1)
                epi = None
                for sc in range(NCH):
                    if sc + 2 < NCH:
                        stage(sc + 2)
                    ensure_proj_done((sc, 0))
                    if attn_on:
                        epi = emit_attn_chunk(sc, epi)
                        if sc == HALF - 1 and p12_reps == 1:
                            if epi is not None:
                                epi()
                                epi = None
                            emit_cc(a2a_in1, a2a_out1)
                            emit_at_loads(a2a_out1, at1, "at1")
                if epi is not None:
                    epi()
                drain_all()

            def emit_cc(ab_in, ab_out):
                nc.gpsimd.collective_compute(
                    "AllToAll",
                    mybir.AluOpType.bypass,
                    replica_groups=[list(range(N_CORES))],
                    ins=[ab_in.opt()],
                    outs=[ab_out.opt()],
                )

            def emit_p3(at1):
                if not at1:  # cc1 not fired inside p12 (rep-timing mode)
                    emit_cc(a2a_in1, a2a_out1)
                    emit_at_loads(a2a_out1, at1, "at1")
                emit_cc(a2a_in2, a2a_out2)
                at2 = []
                emit_at_loads(a2a_out2, at2, "at2")
                for i in range(N_CORES):
                    nc.gpsimd.tensor_add(at1[i][:], at1[i][:], at2[i][:])
                for e in range(KD):
                    pso = pspool.tile([P, SW], F32, tag="mm",
                                      name=f"pso_{e}")
                    for i in range(N_CORES):
                        nc.tensor.matmul(
                            pso[:],
                            wot[e][:, bass.ts(i, 128)],
                            at1[i][:],
                            start=(i == 0),
                            stop=(i == N_CORES - 1),
                        )
                    ot = ptpool.tile([P, SW], F32, tag="ot", name=f"ot_{e}")
                    nc.vector.tensor_copy(ot[:], pso[:])
                    nc.sync.dma_start(out=out_d[bass.ts(e, 128)], in_=ot[:])

            at1 = []
            if p12_reps == 1:
                staged0 = emit_proj_dmas(0)
                emit_consts()
                emit_p12(at1, staged0)
            else:
                emit_consts()
                with tc.For_i(0, p12_reps, 1):
                    emit_p12(at1)
            for r3 in range(p3_reps):
                emit_p3(at1 if r3 == 0 else [])

    nc.finalize()
    return nc


def prepare_in_maps(in_features, token_positions, Wq, Wk, Wv, Wo, seq):
    """Host-side staging: shard/transform full inputs into per-core maps."""
    import ml_dtypes
    adt = ml_dtypes.bfloat16
    x = np.ascontiguousarray(np.asarray(in_features, dtype=np.float32)[0])
    pos = np.asarray(token_positions).reshape(-1)[:seq].astype(np.float64)

    xt = np.ascontiguousarray(x.T)  # [D, S]

    # RoPE tables in rotate-half form after pair permutation.
    inv_freq = THETA ** (-np.arange(0, HEAD_DIM, 2, dtype=np.float64)
                         / HEAD_DIM)
    ang = pos[:, None] * inv_freq[None, :]  # [S, 32]
    cos = np.cos(ang).T.astype(np.float32)  # [32, S]
    sin = np.sin(ang).T.astype(np.float32)
    ctab = np.ascontiguousarray(np.tile(cos, (4, 1)))  # [128, S]
    stab = np.ascontiguousarray(
        np.concatenate([-sin, sin, -sin, sin], axis=0)
    ).astype(np.float32)

    perm = np.concatenate(
        [np.arange(0, HEAD_DIM, 2), np.arange(1, HEAD_DIM, 2)]
    )  # within-head: evens then odds

    CH = min(512, seq)
    KBC = CH // 128
    tri = np.triu(np.ones((128, 128), dtype=np.float32))
    dmask = np.ones((128, KBC * CH), dtype=np.float32)
    for j in range(KBC):
        dmask[:, CH * j:CH * j + 128 * j] = 0.0
        dmask[:, CH * j + 128 * j:CH * j + 128 * (j + 1)] = tri
    ident = np.eye(128, dtype=np.float32)
    ones = np.ones((128, max(seq // 128, 64)), dtype=np.float32)
    e01_host = np.zeros((2, 128), dtype=np.float32)
    e01_host[0, 0:64] = 1.0
    e01_host[1, 64:128] = 1.0

    WoT = np.ascontiguousarray(np.asarray(Wo, dtype=np.float32).T)  # [d, e]
    wo_packed = np.empty((128, KD * D_MODEL), dtype=np.float32)
    for e in range(KD):
        for i in range(KD):
            wo_packed[:, D_MODEL * e + 128 * i: D_MODEL * e + 128 * (i + 1)] \
                = WoT[128 * i:128 * (i + 1), 128 * e:128 * (e + 1)]

    def pack_w(Wc):
        # Wc: [128 out, 1024 in] -> WT [1024, 128] -> [128, 8*128] k-tiled
        WT = np.ascontiguousarray(Wc.T)
        return np.ascontiguousarray(
            WT.reshape(KD, 128, 128).transpose(1, 0, 2).reshape(128, KD * 128)
        ).astype(np.float32)

    in_maps = []
    for c in range(N_CORES):
        rows = slice(128 * c, 128 * (c + 1))
        Wq_r = np.asarray(Wq, dtype=np.float32)[rows].reshape(2, 64, D_MODEL)
        Wq_c = (Wq_r[:, perm, :] / math.sqrt(HEAD_DIM)).reshape(128, D_MODEL)
        Wk_r = np.asarray(Wk, dtype=np.float32)[rows].reshape(2, 64, D_MODEL)
        Wk_c = Wk_r[:, perm, :].reshape(128, D_MODEL)
        Wv_c = np.asarray(Wv, dtype=np.float32)[rows]
        in_maps.append({
            "xt": xt.astype(adt),
            "wq": pack_w(Wq_c).astype(adt),
            "wk": pack_w(Wk_c).astype(adt),
            "wv": pack_w(Wv_c).astype(adt),
            "wo": wo_packed.astype(adt),
            "ctab": ctab,
            "stab": stab,
            "dmask": dmask.astype(adt),
            "ident": ident,
            "ones": ones.astype(adt),
            "zeros": np.zeros((128, seq // 8), dtype=adt),
            "e01": e01_host,
        })
    return in_maps


_BUILD_CACHE = {}


def _get_nc(seq, p12_reps=1, p3_reps=1, parts="full"):
    key = (seq, p12_reps, p3_reps, parts)
    if key not in _BUILD_CACHE:
        _BUILD_CACHE[key] = build(seq, p12_reps, p3_reps, parts)
    return _BUILD_CACHE[key]


def postprocess(results, seq, in_dtype):
    SW = seq // N_CORES
    out = np.empty((seq, D_MODEL), dtype=np.float32)
    for c in range(N_CORES):
        out[SW * c:SW * (c + 1), :] = results[c]["out"].T
    return out.reshape(1, seq, D_MODEL).astype(in_dtype)


def kernel(in_features, token_positions, Wq, Wk, Wv, Wo):
    in_dtype = np.asarray(in_features).dtype
    B, S, D = np.asarray(in_features).shape
    assert B == 1 and D == D_MODEL

    nc = _get_nc(S)
    in_maps = prepare_in_maps(in_features, token_positions, Wq, Wk, Wv, Wo, S)
    res = run_bass_kernel_spmd(nc, in_maps, list(range(N_CORES)), trace=False)
    return postprocess(res.results, S, in_dtype)


# revision 39
# speedup vs baseline: 1.1653x; 1.1653x over previous
"""Multi-head self-attention with RoPE, sharded over 8 TRN2 NeuronCores.

Sharding: tensor-parallel over heads (2 heads/core) for QKV projections and
attention; an AllToAll redistributes attention outputs from head-sharded to
sequence-sharded so each core computes 1/8 of the output projection rows.

Device-side layout choices (host pre-stages everything):
- x is passed transposed (xt = x.T) so projection matmuls contract naturally.
- Wq/Wk rows are pair-permuted (evens then odds per head) so RoPE becomes
  rotate-half form; the 1/sqrt(hd) score scale is folded into Wq.
- Scores are computed transposed (S^T = K @ Q^T, keys on partitions) so the
  softmax denominator comes free from an ones-column appended to V, and P^T
  feeds the PV matmul with no on-device transpose of P.
- Attention outputs are normalized per chunk BEFORE the AllToAll (approx
  reciprocal + e01-broadcast matmul), so the collective payload is a single
  contiguous bf16 [128, 512] per shard and the consumer side needs no
  normalization stage.
- proj+rope of chunk sc+1 is emitted before attention of chunk sc so the
  DVE rope work overlaps the PE/ACT attention work of the previous chunk.
- All matmuls run as bf16/float32r (full PE rate).

Hardcoded problem shape: B=1, S=4096, D=1024, H=16, hd=64, theta=10000.
"""

import math

import numpy as np

import concourse.bass as bass
import concourse.mybir as mybir
import concourse.tile as tile
from concourse import bacc
from concourse.bass_utils import run_bass_kernel_spmd

N_CORES = 8
D_MODEL = 1024
NUM_HEADS = 16
HEAD_DIM = 64
THETA = 10000.0
P = 128  # partitions; also = 2 heads x 64 dims per core
KD = D_MODEL // 128  # 8 contraction tiles for the projections

F32 = mybir.dt.float32
F32R = mybir.dt.float32r
BF16 = mybir.dt.bfloat16
EXP = mybir.ActivationFunctionType.Exp

ADT = BF16  # attention matmul dtype (x, Wqkv, Q/K, V, P)


def build(seq: int, p12_reps: int = 1, p3_reps: int = 1, parts: str = "full"):
    """Build the SPMD Bass program for sequence length `seq`.

    p12_reps > 1 wraps phases 1+2 (projections + attention) in an on-device
    For_i loop; p3_reps > 1 unrolls phase 3 (A2A + out-proj) — both exist
    for wall-clock timing above the axon dispatch floor. Defaults give the
    normal single-shot kernel.
    """
    CH = min(512, seq)          # free-dim chunk for matmuls / PSUM banks
    NCH = seq // CH             # number of seq chunks
    KB = seq // 128             # key blocks
    KBC = CH // 128             # key blocks per chunk (4 at CH=512)
    SW = seq // N_CORES         # per-core output seq shard
    HALF = N_CORES // 2

    nc = bacc.Bacc("TRN2", num_devices=N_CORES)

    xt = nc.dram_tensor("xt", [D_MODEL, seq], ADT, kind="ExternalInput")
    wq = nc.dram_tensor("wq", [P, D_MODEL], ADT, kind="ExternalInput")
    wk = nc.dram_tensor("wk", [P, D_MODEL], ADT, kind="ExternalInput")
    wv = nc.dram_tensor("wv", [P, D_MODEL], ADT, kind="ExternalInput")
    wo = nc.dram_tensor("wo", [P, KD * D_MODEL], BF16, kind="ExternalInput")
    ctab = nc.dram_tensor("ctab", [P, seq], F32, kind="ExternalInput")
    stab = nc.dram_tensor("stab", [P, seq], F32, kind="ExternalInput")
    dmaskd = nc.dram_tensor("dmask", [P, KBC * CH], BF16,
                            kind="ExternalInput")
    ident = nc.dram_tensor("ident", [P, 128], F32, kind="ExternalInput")
    onesd = nc.dram_tensor("ones", [P, max(KB, 64)], ADT, kind="ExternalInput")
    e01d = nc.dram_tensor("e01", [2, P], F32R, kind="ExternalInput")
    zerod = nc.dram_tensor("zeros", [P, SW], BF16, kind="ExternalInput")
    out_d = nc.dram_tensor("out", [D_MODEL, SW], F32, kind="ExternalOutput")

    with tile.TileContext(nc) as tc:
        with (
            tc.tile_pool(name="const", bufs=1) as cpool,
            tc.tile_pool(name="mats", bufs=1) as mpool,
            tc.tile_pool(name="xt", bufs=2) as xpool,
            tc.tile_pool(name="sc", bufs=2) as spool,
            tc.tile_pool(name="pt", bufs=6) as ptpool,
            tc.tile_pool(name="at", bufs=1) as atpool,
            tc.tile_pool(name="ps", bufs=2, space="PSUM") as pspool,
            tc.tile_pool(name="pss", bufs=2, space="PSUM") as psspool,
            tc.tile_pool(name="dram", bufs=1, space="DRAM") as dpool,
        ):
            # ---- projection weights first: chunk 0 depends on them ----
            w_sb = {}
            for name, src in (("q", wq), ("k", wk), ("v", wv)):
                t = cpool.tile([P, D_MODEL], ADT, tag=f"w{name}")
                nc.sync.dma_start(out=t[:], in_=src[:])
                w_sb[name] = t
            dmask = idn = ones = e01 = None
            wot = []

            def emit_consts():
                """Remaining constants — emitted after chunk 0's input DMAs
                so they don't delay the first projection."""
                nonlocal dmask, idn, ones, e01
                dmask = cpool.tile([P, KBC * CH], BF16, tag="dmask")
                nc.sync.dma_start(out=dmask[:], in_=dmaskd[:])
                idn = cpool.tile([P, 128], F32, tag="idn")
                nc.sync.dma_start(out=idn[:], in_=ident[:])
                ones = cpool.tile([P, max(KB, 64)], ADT, tag="ones")
                nc.sync.dma_start(out=ones[:], in_=onesd[:])
                e01 = {}
                for h in (0, 1):
                    t = cpool.tile([1, P], F32R, tag=f"e01{h}")
                    nc.sync.dma_start(out=t[:], in_=e01d[h:h + 1, :])
                    e01[h] = t
                for e in range(KD):
                    t = cpool.tile([P, D_MODEL], BF16, tag=f"wo{e}")
                    nc.sync.dma_start(out=t[:], in_=wo[:, bass.ts(e, D_MODEL)])
                    wot.append(t)
                # zero the never-written a2a halves once (DRAM->DRAM)
                for s_ in range(N_CORES):
                    dst = a2a_in1 if s_ >= HALF else a2a_in2
                    nc.sync.dma_start(out=dst[s_], in_=zerod[:])
                # ones columns of vnat (cols 64 and 129 of each 130-block)
                vv = vnat[:].rearrange("p (k c) -> p k c", c=130)
                oo = ones[:, 0:KB].rearrange("p (k c) -> p k c", c=1)
                nc.vector.tensor_copy(vv[:, :, 64:65], oo)
                nc.vector.tensor_copy(vv[:, :, 129:130], oo)

            # ---- persistent matrices ----
            qT = mpool.tile([P, seq], ADT, tag="qT")  # rows: 2 heads x 64
            kT = mpool.tile([P, seq], ADT, tag="kT")
            vnat = mpool.tile([P, KB * 130], ADT, tag="vnat")

            a2a_in1 = dpool.tile([N_CORES, P, SW], BF16, tag="a2a_in1")
            a2a_out1 = dpool.tile([N_CORES, P, SW], BF16, tag="a2a_out1")
            a2a_in2 = dpool.tile([N_CORES, P, SW], BF16, tag="a2a_in2")
            a2a_out2 = dpool.tile([N_CORES, P, SW], BF16, tag="a2a_out2")

            def emit_proj_dmas(sc):
                """Issue the input DMAs for chunk sc; returns the tiles."""
                sl = bass.ts(sc, CH)
                xts = []
                for k in range(KD):
                    t = xpool.tile([P, CH], ADT, tag=f"xt{k}",
                                   name=f"xt_{sc}_{k}")
                    nc.sync.dma_start(
                        out=t[:], in_=xt[128 * k:128 * (k + 1), sl]
                    )
                    xts.append(t)
                ct_c = spool.tile([P, CH], F32, tag="ct", name=f"ct_{sc}")
                nc.sync.dma_start(out=ct_c[:], in_=ctab[:, sl])
                st_c = spool.tile([P, CH], F32, tag="st", name=f"st_{sc}")
                nc.sync.dma_start(out=st_c[:], in_=stab[:, sl])
                return xts, ct_c, st_c

            def proj_qk_units(sc, staged):
                """Generator of PE-op units for chunk sc's Q/K projections +
                rope, interleavable between attention blocks. DVE ops are
                emitted inline at their dependency points."""
                sl = bass.ts(sc, CH)
                xts, ct_c, st_c = staged
                if parts == "dma":
                    return
                sw_c = {}
                for nm in ("qs", "ks"):
                    sw_c[nm] = spool.tile([P, CH], F32, tag=nm,
                                          name=f"sw_{sc}_{nm}")
                for name, dst in (("q", qT[:, sl]), ("k", kT[:, sl])):
                    ps = pspool.tile([P, CH], F32, tag="mm",
                                     name=f"proj_{sc}_{name}")
                    for k in range(KD):
                        nc.tensor.matmul(
                            ps[:],
                            w_sb[name][:, bass.ts(k, 128)],
                            xts[k][:],
                            start=(k == 0),
                            stop=(k == KD - 1),
                        )
                        if k % 2 == 1:
                            yield
                    nc.vector.tensor_copy(dst, ps[:])
                    if parts == "proj":
                        continue
                    # rope: mat = mat*cos + swapped*sin (swapped halves via
                    # DVE copies)
                    mat = qT if name == "q" else kT
                    eng = nc.vector
                    swc = sw_c["qs" if name == "q" else "ks"]
                    for h in (0, 1):
                        for half in (0, 1):
                            d0 = 64 * h + 32 * half
                            s0 = 64 * h + 32 * (1 - half)
                            eng.tensor_copy(
                                swc[d0:d0 + 32, :], mat[s0:s0 + 32, sl]
                            )
                    tm = spool.tile([P, CH], F32, tag=f"tmp_{name}",
                                    name=f"tmp_{sc}_{name}")
                    eng.tensor_mul(tm[:], swc[:], st_c[:])
                    eng.tensor_mul(mat[:, sl], mat[:, sl], ct_c[:])
                    eng.tensor_add(mat[:, sl], mat[:, sl], tm[:])

            def proj_v_units(sc, staged):
                """V projection + per-block transpose into vnat for chunk sc
                (needed only by chunk sc's own diagonal PV, so it drains
                later than the Q/K units)."""
                xts, _, _ = staged
                if parts == "dma":
                    return
                vt_c = spool.tile([P, CH], F32, tag="vt", name=f"vt_{sc}")
                ps = pspool.tile([P, CH], F32, tag="mm",
                                 name=f"proj_{sc}_v")
                for k in range(KD):
                    nc.tensor.matmul(
                        ps[:],
                        w_sb["v"][:, bass.ts(k, 128)],
                        xts[k][:],
                        start=(k == 0),
                        stop=(k == KD - 1),
                    )
                    if k % 2 == 1:
                        yield
                nc.vector.tensor_copy(vt_c[:], ps[:])
                if parts in ("proj", "rope"):
                    return
                for j in range(KBC):
                    kb = sc * KBC + j
                    pst = pspool.tile([P, CH], F32, tag="mm",
                                      name=f"vtr_{kb}")
                    nc.tensor.transpose(
                        pst[:, 0:128], vt_c[:, bass.ts(j, 128)], idn[:]
                    )
                    nc.vector.tensor_copy(
                        vnat[:, 130 * kb:130 * kb + 64], pst[:, 0:64]
                    )
                    nc.vector.tensor_copy(
                        vnat[:, 130 * kb + 65:130 * kb + 129],
                        pst[:, 64:128]
                    )
                    yield

            # Global queue of ((chunk, kind), proj-unit generator) in need
            # order: kind 0 = Q/K+rope (needed when attn(chunk) starts),
            # kind 1 = V+transpose (needed at attn(chunk)'s diagonal).
            # Attention blocks pump one PE unit per block so projection work
            # spreads evenly instead of clumping at chunk boundaries.
            proj_queue = []

            def pump(n=1):
                while n > 0 and proj_queue:
                    try:
                        next(proj_queue[0][1])
                        n -= 1
                    except StopIteration:
                        proj_queue.pop(0)

            def ensure_proj_done(key):
                while proj_queue and proj_queue[0][0] <= key:
                    for _ in proj_queue[0][1]:
                        pass
                    proj_queue.pop(0)

            def drain_all():
                ensure_proj_done((NCH, 1))

            def emit_attn_chunk(qc, prev_epi=None):
                """Attention for query chunk qc (needs proj chunks 0..qc).

                Per key block: S^T for both heads lands in one [128, 2*CH]
                PSUM tile ([0:CH]=h0, [CH:2CH]=h1) so a single wide exp
                covers both heads. Diagonal-band blocks are processed FIRST
                (their masking runs on GPSIMD and gets latency-hidden behind
                the non-diagonal tail of the PV accumulation). Chunk qc+1's
                projection PE units (`feeder`) are pumped between blocks to
                fill TensorE slack while ScalarE streams the exps.
                """
                kbmax = (qc + 1) * KBC
                psu = {}
                for h in (0, 1):
                    psu[h] = pspool.tile([65, CH], F32, tag="u",
                                         name=f"psu_{qc}_{h}")
                # old blocks first: they only need this chunk's roped Q, so
                # attention can start before rope-K of this chunk finishes;
                # the diagonal band comes last (its GPSIMD masks still hide
                # behind the deferred-PV pipeline)
                kb_order = (list(range(0, kbmax - KBC))
                            + list(range(kbmax - KBC, kbmax)))

                def emit_pv(kb, ki, pt, qoff):
                    for h in (0, 1):
                        nc.tensor.matmul(
                            psu[h][:, qoff:CH],
                            vnat[:, 130 * kb + 65 * h:
                                 130 * kb + 65 * (h + 1)],
                            pt[:, CH * h + qoff:CH * (h + 1)],
                            start=(ki == 0),
                            stop=(ki == kbmax - 1),
                        )

                pending = None  # deferred-one-step PV: (kb, ki, pt, qoff)
                for ki, kb in enumerate(kb_order):
                    j = kb - (kbmax - KBC)  # diag index if >= 0
                    if j == 0:
                        # own-chunk V/vnat needed from the first diag PV on
                        ensure_proj_done((qc, 1))
                    # diag block j only contributes to queries >= 128j; skip
                    # the fully-masked left region in scores, exp, and PV
                    qoff = 128 * j if j > 0 else 0
                    pss = psspool.tile([P, 2 * CH], F32, tag="s",
                                       name=f"sc_{qc}_{kb}")
                    for h in (0, 1):
                        nc.tensor.matmul(
                            pss[:, CH * h + qoff:CH * (h + 1)],
                            kT[64 * h:64 * (h + 1), bass.ts(kb, 128)],
                            qT[64 * h:64 * (h + 1),
                               CH * qc + qoff:CH * (qc + 1)],
                            start=True,
                            stop=True,
                        )
                    if parts == "attn_sc":
                        pump()
                        continue
                    pt = ptpool.tile([P, 2 * CH], ADT, tag="pt",
                                     name=f"pt_{qc}_{kb}")
                    if qoff == 0:
                        nc.scalar.activation(pt[:], pss[:], EXP)
                    else:
                        for h in (0, 1):
                            nc.scalar.activation(
                                pt[:, CH * h + qoff:CH * (h + 1)],
                                pss[:, CH * h + qoff:CH * (h + 1)], EXP)
                    if j >= 0:
                        # triangle mask on the [128j, 128j+128) query range
                        for h in (0, 1):
                            nc.gpsimd.tensor_mul(
                                pt[:, CH * h + 128 * j:
                                   CH * h + 128 * (j + 1)],
                                pt[:, CH * h + 128 * j:
                                   CH * h + 128 * (j + 1)],
                                dmask[:, CH * j + 128 * j:
                                      CH * j + 128 * (j + 1)],
                            )
                    if parts == "attn_s":
                        pump()
                        if ki == 1 and prev_epi is not None:
                            prev_epi()
                            prev_epi = None
                        continue
                    if pending is not None:
                        emit_pv(*pending)
                    pending = (kb, ki, pt, qoff)
                    pump()
                    if ki == 1 and prev_epi is not None:
                        # previous chunk's epilogue, deferred past this
                        # chunk's first scores so ScalarE never starves on
                        # the sums->reciprocal->broadcast chain
                        prev_epi()
                        prev_epi = None
                if prev_epi is not None:
                    prev_epi()
                if parts not in ("attn_s", "attn_sc") and pending is not None:
                    emit_pv(*pending)
                if parts in ("attn_s", "attn_sc", "attn_pv"):
                    return None

                def epilogue():
                    # normalize U by the softmax sums (row 64 of each psu)
                    # and ship one contiguous bf16 [128, SW] per shard; the
                    # psu-releasing copies come first
                    from concourse.dve_ops import (
                        RECIP_APPROX_FAST_CONSTS as _RC,
                        RECIPROCAL_APPROX_FAST as _RA,
                    )
                    sms = []
                    for h in (0, 1):
                        sm = spool.tile([1, CH], F32, tag=f"sm{h}",
                                        name=f"sm_{qc}_{h}")
                        nc.vector.tensor_copy(sm[:], psu[h][64:65, :])
                        sms.append(sm)
                    uu = spool.tile([P, CH], F32, tag="uu", name=f"uu_{qc}")
                    for h in (0, 1):
                        nc.vector.tensor_copy(
                            uu[64 * h:64 * (h + 1), :], psu[h][0:64, :]
                        )
                    rb = pspool.tile([P, CH], F32, tag="mm", name=f"rb_{qc}")
                    for h in (0, 1):
                        rs = spool.tile([1, CH], F32R, tag=f"rs{h}",
                                        name=f"rs_{qc}_{h}")
                        nc.vector._custom_dve(
                            _RA, out=rs[:], in0=sms[h][:],
                            s0=_RC["s0"], s1=_RC["s1"], imm2=_RC["imm2"],
                        )
                        nc.tensor.matmul(rb[:], e01[h][:], rs[:],
                                         start=(h == 0), stop=(h == 1))
                    ut = ptpool.tile([P, CH], BF16, tag="ut", name=f"ut_{qc}")
                    nc.vector.tensor_mul(ut[:], uu[:], rb[:])
                    tgt = a2a_in1 if qc < HALF else a2a_in2
                    nc.sync.dma_start(out=tgt[qc], in_=ut[:])

                return epilogue

            def emit_at_loads(ab_out, tiles, tag):
                for i in range(N_CORES):
                    at = atpool.tile([P, SW], BF16, tag=f"{tag}{i}",
                                     name=f"{tag}_{i}")
                    nc.sync.dma_start(out=at[:], in_=ab_out[i])
                    tiles.append(at)

            def emit_p12(at1, staged0=None):
                # proj is staged 2 chunks ahead of attention; its PE units
                # go through the global proj_queue, pumped one per attention
                # block so projection work spreads evenly over the phase and
                # the ScalarE exp stream stays dense.
                attn_on = parts not in ("dma", "proj", "rope")

                def stage(sc):
                    staged = emit_proj_dmas(sc)
                    proj_queue.append(((sc, 0), proj_qk_units(sc, staged)))
                    proj_queue.append(((sc, 1), proj_v_units(sc, staged)))

                if staged0 is not None:
                    proj_queue.append(((0, 0), proj_qk_units(0, staged0)))
                    proj_queue.append(((0, 1), proj_v_units(0, staged0)))
                else:
                    stage(0)
                if NCH > 1:
                    stage(1)
                epi = None
                for sc in range(NCH):
                    if sc + 2 < NCH:
                        stage(sc + 2)
                    ensure_proj_done((sc, 0))
                    if attn_on:
                        epi = emit_attn_chunk(sc, epi)
                        if sc == HALF - 1 and p12_reps == 1:
                            if epi is not None:
                                epi()
                                epi = None
                            emit_cc(a2a_in1, a2a_out1)
                            emit_at_loads(a2a_out1, at1, "at1")
                if epi is not None:
                    epi()
                drain_all()

            def emit_cc(ab_in, ab_out):
                nc.gpsimd.collective_compute(
                    "AllToAll",
                    mybir.AluOpType.bypass,
                    replica_groups=[list(range(N_CORES))],
                    ins=[ab_in.opt()],
                    outs=[ab_out.opt()],
                )

            def emit_p3(at1):
                if not at1:  # cc1 not fired inside p12 (rep-timing mode)
                    emit_cc(a2a_in1, a2a_out1)
                    emit_at_loads(a2a_out1, at1, "at1")
                emit_cc(a2a_in2, a2a_out2)
                at2 = []
                emit_at_loads(a2a_out2, at2, "at2")
                for i in range(N_CORES):
                    eng = nc.gpsimd if i % 2 else nc.vector
                    eng.tensor_add(at1[i][:], at1[i][:], at2[i][:])
                for e in range(KD):
                    pso = pspool.tile([P, SW], F32, tag="mm",
                                      name=f"pso_{e}")
                    for i in range(N_CORES):
                        nc.tensor.matmul(
                            pso[:],
                            wot[e][:, bass.ts(i, 128)],
                            at1[i][:],
                            start=(i == 0),
                            stop=(i == N_CORES - 1),
                        )
                    ot = ptpool.tile([P, SW], F32, tag="ot", name=f"ot_{e}")
                    nc.vector.tensor_copy(ot[:], pso[:])
                    nc.sync.dma_start(out=out_d[bass.ts(e, 128)], in_=ot[:])

            at1 = []
            if p12_reps == 1:
                staged0 = emit_proj_dmas(0)
                emit_consts()
                emit_p12(at1, staged0)
            else:
                emit_consts()
                with tc.For_i(0, p12_reps, 1):
                    emit_p12(at1)
            for r3 in range(p3_reps):
                emit_p3(at1 if r3 == 0 else [])

    nc.finalize()
    return nc


def prepare_in_maps(in_features, token_positions, Wq, Wk, Wv, Wo, seq):
    """Host-side staging: shard/transform full inputs into per-core maps."""
    import ml_dtypes
    adt = ml_dtypes.bfloat16
    x = np.ascontiguousarray(np.asarray(in_features, dtype=np.float32)[0])
    pos = np.asarray(token_positions).reshape(-1)[:seq].astype(np.float64)

    xt = np.ascontiguousarray(x.T)  # [D, S]

    # RoPE tables in rotate-half form after pair permutation.
    inv_freq = THETA ** (-np.arange(0, HEAD_DIM, 2, dtype=np.float64)
                         / HEAD_DIM)
    ang = pos[:, None] * inv_freq[None, :]  # [S, 32]
    cos = np.cos(ang).T.astype(np.float32)  # [32, S]
    sin = np.sin(ang).T.astype(np.float32)
    ctab = np.ascontiguousarray(np.tile(cos, (4, 1)))  # [128, S]
    stab = np.ascontiguousarray(
        np.concatenate([-sin, sin, -sin, sin], axis=0)
    ).astype(np.float32)

    perm = np.concatenate(
        [np.arange(0, HEAD_DIM, 2), np.arange(1, HEAD_DIM, 2)]
    )  # within-head: evens then odds

    CH = min(512, seq)
    KBC = CH // 128
    tri = np.triu(np.ones((128, 128), dtype=np.float32))
    dmask = np.ones((128, KBC * CH), dtype=np.float32)
    for j in range(KBC):
        dmask[:, CH * j:CH * j + 128 * j] = 0.0
        dmask[:, CH * j + 128 * j:CH * j + 128 * (j + 1)] = tri
    ident = np.eye(128, dtype=np.float32)
    ones = np.ones((128, max(seq // 128, 64)), dtype=np.float32)
    e01_host = np.zeros((2, 128), dtype=np.float32)
    e01_host[0, 0:64] = 1.0
    e01_host[1, 64:128] = 1.0

    WoT = np.ascontiguousarray(np.asarray(Wo, dtype=np.float32).T)  # [d, e]
    wo_packed = np.empty((128, KD * D_MODEL), dtype=np.float32)
    for e in range(KD):
        for i in range(KD):
            wo_packed[:, D_MODEL * e + 128 * i: D_MODEL * e + 128 * (i + 1)] \
                = WoT[128 * i:128 * (i + 1), 128 * e:128 * (e + 1)]

    def pack_w(Wc):
        # Wc: [128 out, 1024 in] -> WT [1024, 128] -> [128, 8*128] k-tiled
        WT = np.ascontiguousarray(Wc.T)
        return np.ascontiguousarray(
            WT.reshape(KD, 128, 128).transpose(1, 0, 2).reshape(128, KD * 128)
        ).astype(np.float32)

    in_maps = []
    for c in range(N_CORES):
        rows = slice(128 * c, 128 * (c + 1))
        Wq_r = np.asarray(Wq, dtype=np.float32)[rows].reshape(2, 64, D_MODEL)
        Wq_c = (Wq_r[:, perm, :] / math.sqrt(HEAD_DIM)).reshape(128, D_MODEL)
        Wk_r = np.asarray(Wk, dtype=np.float32)[rows].reshape(2, 64, D_MODEL)
        Wk_c = Wk_r[:, perm, :].reshape(128, D_MODEL)
        Wv_c = np.asarray(Wv, dtype=np.float32)[rows]
        in_maps.append({
            "xt": xt.astype(adt),
            "wq": pack_w(Wq_c).astype(adt),
            "wk": pack_w(Wk_c).astype(adt),
            "wv": pack_w(Wv_c).astype(adt),
            "wo": wo_packed.astype(adt),
            "ctab": ctab,
            "stab": stab,
            "dmask": dmask.astype(adt),
            "ident": ident,
            "ones": ones.astype(adt),
            "zeros": np.zeros((128, seq // 8), dtype=adt),
            "e01": e01_host,
        })
    return in_maps


_BUILD_CACHE = {}


def _get_nc(seq, p12_reps=1, p3_reps=1, parts="full"):
    key = (seq, p12_reps, p3_reps, parts)
    if key not in _BUILD_CACHE:
        _BUILD_CACHE[key] = build(seq, p12_reps, p3_reps, parts)
    return _BUILD_CACHE[key]


def postprocess(results, seq, in_dtype):
    SW = seq // N_CORES
    out = np.empty((seq, D_MODEL), dtype=np.float32)
    for c in range(N_CORES):
        out[SW * c:SW * (c + 1), :] = results[c]["out"].T
    return out.reshape(1, seq, D_MODEL).astype(in_dtype)


def kernel(in_features, token_positions, Wq, Wk, Wv, Wo):
    in_dtype = np.asarray(in_features).dtype
    B, S, D = np.asarray(in_features).shape
    assert B == 1 and D == D_MODEL

    nc = _get_nc(S)
    in_maps = prepare_in_maps(in_features, token_positions, Wq, Wk, Wv, Wo, S)
    res = run_bass_kernel_spmd(nc, in_maps, list(range(N_CORES)), trace=False)
    return postprocess(res.results, S, in_dtype)
